# revision 1
# baseline (speedup 1.0000x reference)
"""Sliding-window causal self-attention (GQA + RoPE + RMS-norm + value-embedding
gate) for Trainium2, sharded over 8 NeuronCores.

Sharding: sequence-parallel. (batch=2) x (4 sequence chunks of 1024) = 8 shards.
Each core computes attention for its own 1024 query rows. Window size = 1024 and
chunk size = 1024, so each core only needs K/V for its own chunk plus the
previous 1024 positions (halo). K/V (+rope/rms/gate) are recomputed locally for
the halo instead of communicated -> zero collectives. Chunk-0 shards get a
zero-padded halo; padded keys produce k=0 => exp(score)=0+... exp(0)=1 which is
corrected exactly by subtracting the per-row pad count from the softmax
denominator (padded v rows are 0 so the numerator is untouched).

Key kernel trick: scores are computed pre-transposed (s[k,q] via lhsT=kT,
rhs=qT) so the exp output (bf16) is directly the lhsT of the PV matmul, and V is
augmented with a ones column so the PV matmul emits y[q, 0:128] AND the softmax
denominator Z = y[q, 128] in one accumulation group. Normalization 1/Z is then a
native per-partition tensor_scalar in the natural q-layout.
"""

import math
import sys

import numpy as np

sys.path.insert(0, "/opt/trn_rl_repo")

import ml_dtypes

import concourse.bass as bass
import concourse.bacc as bacc
import concourse.tile as tile
from concourse import mybir
from concourse import bass_utils

BF16 = ml_dtypes.bfloat16
F32 = np.float32

B, T, C = 2, 4096, 1024
H, HKV, D = 8, 2, 128
REP = H // HKV
WIN = 1024
RCHUNK = 1024          # own rows per core
E = 2048               # ext rows (halo + own)
NRT = E // 128         # 16 ext row tiles
NQT = RCHUNK // 128    # 8 q tiles
NKC = 9                # k chunks per q tile
NCT = C // 128         # 8 contraction tiles
EPS = float(np.finfo(np.float32).eps)
SCALE = 1.0 / math.sqrt(D)
NEG = -1.0e30

dt = mybir.dt
AF = mybir.ActivationFunctionType
ALU = mybir.AluOpType
AX = mybir.AxisListType


def _bcast(ap, n, axis_pos=1):
    """Insert a 0-stride dim of size n into an AP at free-axis position."""
    new_ap = list(ap.ap)
    new_ap.insert(axis_pos, [0, n])
    return bass.AP(tensor=ap.tensor, offset=ap.offset, ap=new_ap)


def _halfswap(ap, nh):
    """View [128, nh, 128] with the two 64-wide halves of the last dim
    swapped: out[p, h, 0:64] = in[p, h, 64:128] and vice versa."""
    elem = ap.tensor.dtype_size() if hasattr(ap.tensor, "dtype_size") else None
    base = list(ap.ap)
    # base is [[pstep,128],[hstep,nh],[1,128]] after slicing; rebuild last dim
    return bass.AP(tensor=ap.tensor, offset=ap.offset + 64,
                   ap=[base[0], base[1], [-64, 2], [1, 64]])


def build_nc():
    nc = bacc.Bacc("TRN2", target_bir_lowering=False, debug=False)

    xT_d = nc.dram_tensor("xT", [C, E], dt.bfloat16, kind="ExternalInput").ap()
    wq_d = nc.dram_tensor("wq", [C, C], dt.bfloat16, kind="ExternalInput").ap()
    wkv_d = nc.dram_tensor("wkv", [C, 512], dt.bfloat16, kind="ExternalInput").ap()
    wo_d = nc.dram_tensor("wo", [C, C], dt.bfloat16, kind="ExternalInput").ap()
    wg_d = nc.dram_tensor("wg", [32, HKV], dt.bfloat16, kind="ExternalInput").ap()
    ve_d = nc.dram_tensor("ve2", [E, HKV * D], dt.bfloat16, kind="ExternalInput").ap()
    cs_d = nc.dram_tensor("cs", [E, 256], dt.bfloat16, kind="ExternalInput").ap()
    tri_d = nc.dram_tensor("tri", [128, 2 * 128], dt.bfloat16, kind="ExternalInput").ap()
    npad_d = nc.dram_tensor("npad", [128, NQT], dt.float32, kind="ExternalInput").ap()
    id_d = nc.dram_tensor("ident", [128, 128], dt.bfloat16, kind="ExternalInput").ap()
    out_d = nc.dram_tensor("out", [RCHUNK, C], dt.float32, kind="ExternalOutput").ap()

    with tile.TileContext(nc) as tc:
        _body(tc, xT_d, wq_d, wkv_d, wo_d, wg_d, ve_d, cs_d, tri_d, npad_d, id_d,
              out_d)
    nc.compile()
    return nc


def _body(tc, xT_d, wq_d, wkv_d, wo_d, wg_d, ve_d, cs_d, tri_d, npad_d, id_d,
          out_d):
    nc = tc.nc
    from contextlib import ExitStack

    with ExitStack() as ctx:
        const = ctx.enter_context(tc.tile_pool(name="const", bufs=1))
        persist = ctx.enter_context(tc.tile_pool(name="persist", bufs=1))
        work = ctx.enter_context(tc.tile_pool(name="work", bufs=3))

        # ---- constants / persistent SBUF ----
        # (emitted in consumption order: B1 needs wkv/wg/cs/ve/xT first)
        wkv_sb = const.tile([128, NCT, 512], dt.bfloat16)
        wg_sb = const.tile([32, HKV], dt.bfloat16)
        cs_sb = const.tile([128, NRT, 256], dt.bfloat16)
        ve_sb = const.tile([128, NRT, HKV * D], dt.bfloat16)
        wq_sb = const.tile([128, NCT, C], dt.bfloat16)
        tri_sb = const.tile([128, 2, 128], dt.bfloat16)
        npad_sb = const.tile([128, NQT], dt.float32)
        id_sb = const.tile([128, 128], dt.bfloat16)

        kT_sb = persist.tile([128, HKV, NRT, 128], dt.bfloat16)   # [d, kvh, g, k]
        qT_sb = persist.tile([128, H, NQT, 128], dt.bfloat16)     # [d, h, qt, q]
        v_sb = persist.tile([128, NRT, HKV, 129], dt.bfloat16)    # [k, g, kvh, d|1]
        yN_sb = persist.tile([128, NQT, H, 128], dt.bfloat16)     # [q, qt, h, d]
        krot_sb = persist.tile([128, NRT, HKV * D], dt.bfloat16)  # roped k (pre-norm)
        qrot_sb = persist.tile([128, NQT, C], dt.bfloat16)        # roped q (pre-norm)
        msk_sb = persist.tile([128, NRT, HKV], dt.float32)
        msq_sb = persist.tile([128, NQT, H], dt.float32)
        gate_sb = persist.tile([128, NRT, HKV], dt.float32)

        nc.vector.memset(v_sb[:, :, :, 128:129], 1.0)

        # ======== phase B: projections + rope + rms + transposes (fused) ========
        late = ctx.enter_context(tc.tile_pool(name="late", bufs=1))
        wo_sb = late.tile([128, NCT, C], dt.bfloat16)
        eps_sb = const.tile([128, 1], dt.float32)
        nc.vector.memset(eps_sb, EPS)

        with tc.tile_pool(name="xpool", bufs=1) as xpool:
            xT_sb = xpool.tile([128, NCT, E], dt.bfloat16)
            xTv = xT_d.rearrange("(a p) n -> p a n", p=128)
            nc.sync.dma_start(out=xT_sb[:, 0, :], in_=xTv[:, 0, :])
            nc.sync.dma_start(out=wkv_sb,
                              in_=wkv_d.rearrange("(a p) n -> p a n", p=128))
            nc.sync.dma_start(out=wg_sb, in_=wg_d)
            for ct in range(1, NCT):
                nc.sync.dma_start(out=xT_sb[:, ct, :], in_=xTv[:, ct, :])
            nc.sync.dma_start(out=id_sb, in_=id_d)
            nc.sync.dma_start(out=cs_sb,
                              in_=cs_d.rearrange("(a p) n -> p a n", p=128))
            nc.sync.dma_start(out=ve_sb,
                              in_=ve_d.rearrange("(a p) n -> p a n", p=128))
            nc.sync.dma_start(out=wq_sb,
                              in_=wq_d.rearrange("(a p) n -> p a n", p=128))
            nc.sync.dma_start(out=tri_sb,
                              in_=tri_d.rearrange("p (a n) -> p a n", a=2))
            nc.sync.dma_start(out=npad_sb, in_=npad_d)
            nc.sync.dma_start(out=wo_sb,
                              in_=wo_d.rearrange("(a p) n -> p a n", p=128))

            # all gate matmuls + one sigmoid up front (needs only xT ct=0)
            with tc.tile_pool(name="gps", bufs=1, space="PSUM") as gps:
                g_psum = gps.tile([128, NRT * HKV], dt.float32)
                for rt in range(NRT):
                    nc.tensor.matmul(g_psum[:, bass.ts(rt, HKV)],
                                     lhsT=xT_sb[0:32, 0, bass.ts(rt, 128)],
                                     rhs=wg_sb, start=True, stop=True)
                nc.scalar.activation(out=gate_sb.rearrange("p a n -> p (a n)"),
                                     in_=g_psum, func=AF.Sigmoid)

            mainps = ctx.enter_context(ExitStack())
            kvps = mainps.enter_context(
                tc.tile_pool(name="kvps", bufs=2, space="PSUM"))
            qps = mainps.enter_context(
                tc.tile_pool(name="qps", bufs=2, space="PSUM"))
            tp = mainps.enter_context(
                tc.tile_pool(name="tp", bufs=2, space="PSUM"))

            for rt in range(NRT):
                rs = bass.ts(rt, 128)
                # kv projection: psum [128 rows, 256 k | 256 v]
                kv = kvps.tile([128, 512], dt.float32, tag="kv")
                for ct in range(NCT):
                    nc.tensor.matmul(kv, lhsT=xT_sb[:, ct, rs], rhs=wkv_sb[:, ct, :],
                                     start=(ct == 0), stop=(ct == NCT - 1))
                # v = v_raw + gate*ve, straight from psum
                for kvh in range(HKV):
                    nc.vector.scalar_tensor_tensor(
                        out=v_sb[:, rt, kvh, 0:128],
                        in0=ve_sb[:, rt, bass.ts(kvh, 128)],
                        scalar=gate_sb[:, rt, kvh:kvh + 1],
                        in1=kv[:, 256 + kvh * 128:256 + (kvh + 1) * 128],
                        op0=ALU.mult, op1=ALU.add)
                # k rope on gpsimd (3 passes via [c|c] and [-s|s] tables)
                kraw = work.tile([128, HKV * D], dt.bfloat16, tag="kraw")
                nc.scalar.copy(out=kraw, in_=kv[:, 0:256])
                k3 = kraw.rearrange("p (a n) -> p a n", a=HKV)
                kr3 = krot_sb[:, rt, :].rearrange("p (a n) -> p a n", a=HKV)
                ccb = _bcast(cs_sb[:, rt, 0:128], HKV)
                ssb = _bcast(cs_sb[:, rt, 128:256], HKV)
                kc_ = work.tile([128, HKV, 128], dt.bfloat16, tag="t1")
                ks_ = work.tile([128, HKV, 128], dt.bfloat16, tag="t2")
                nc.gpsimd.tensor_mul(kc_, k3, ccb)
                nc.gpsimd.tensor_mul(ks_, k3, ssb)
                nc.gpsimd.tensor_add(kr3, kc_, _halfswap(ks_, HKV))
                # k rms stats from the ROPED values (exact for any cos/sin)
                ksq = work.tile([128, HKV * D], dt.bfloat16, tag="t1")
                nc.scalar.activation(out=ksq, in_=krot_sb[:, rt, :],
                                     func=AF.Square)
                nc.vector.tensor_reduce(
                    out=msk_sb[:, rt, :],
                    in_=ksq.rearrange("p (a n) -> p a n", a=HKV),
                    axis=AX.X, op=ALU.add)
                nc.scalar.activation(out=msk_sb[:, rt, :], in_=msk_sb[:, rt, :],
                                     func=AF.Sqrt, bias=eps_sb, scale=1.0 / D)
                nc.vector.reciprocal(out=msk_sb[:, rt, :], in_=msk_sb[:, rt, :])
                # normalize + transpose -> kT
                for kvh in range(HKV):
                    nc.vector.tensor_scalar_mul(
                        kr3[:, kvh, :], kr3[:, kvh, :], msk_sb[:, rt, kvh:kvh + 1])
                    ktp = tp.tile([128, 128], dt.bfloat16, tag="tp")
                    nc.tensor.transpose(ktp, kr3[:, kvh, :], id_sb)
                    nc.vector.tensor_copy(out=kT_sb[:, kvh, rt, :], in_=ktp)

                # --- q path (own rows only) ---
                if rt >= NRT - NQT:
                    qt = rt - (NRT - NQT)
                    qp = qps.tile([128, C], dt.float32, tag="q")
                    for half in range(2):
                        o = qp[:, bass.ts(half, 512)]
                        for ct in range(NCT):
                            nc.tensor.matmul(
                                o, lhsT=xT_sb[:, ct, rs],
                                rhs=wq_sb[:, ct, bass.ts(half, 512)],
                                start=(ct == 0), stop=(ct == NCT - 1))
                    qraw = work.tile([128, C], dt.bfloat16, tag="qraw")
                    nc.scalar.copy(out=qraw, in_=qp)
                    q3 = qraw.rearrange("p (a n) -> p a n", a=H)
                    qr3 = qrot_sb[:, qt, :].rearrange("p (a n) -> p a n", a=H)
                    ccbq = _bcast(cs_sb[:, rt, 0:128], H)
                    ssbq = _bcast(cs_sb[:, rt, 128:256], H)
                    u1 = work.tile([128, H, 128], dt.bfloat16, tag="u1")
                    u2 = work.tile([128, H, 128], dt.bfloat16, tag="u2")
                    nc.vector.tensor_mul(u1, q3, ccbq)
                    nc.vector.tensor_mul(u2, q3, ssbq)
                    nc.vector.tensor_add(qr3, u1, _halfswap(u2, H))
                    # q rms stats from the ROPED values (exact for any cos/sin)
                    qsq = work.tile([128, C], dt.bfloat16, tag="u1")
                    nc.scalar.activation(out=qsq, in_=qrot_sb[:, qt, :],
                                         func=AF.Square)
                    nc.vector.tensor_reduce(
                        out=msq_sb[:, qt, :],
                        in_=qsq.rearrange("p (a n) -> p a n", a=H),
                        axis=AX.X, op=ALU.add)
                    nc.scalar.activation(out=msq_sb[:, qt, :],
                                         in_=msq_sb[:, qt, :],
                                         func=AF.Sqrt, bias=eps_sb, scale=1.0 / D)
                    nc.vector.reciprocal(out=msq_sb[:, qt, :],
                                         in_=msq_sb[:, qt, :])
                    for h in range(H):
                        nc.vector.tensor_scalar(
                            out=qr3[:, h, :], in0=qr3[:, h, :],
                            scalar1=msq_sb[:, qt, h:h + 1], scalar2=SCALE,
                            op0=ALU.mult, op1=ALU.mult)
                        qtp = tp.tile([128, 128], dt.bfloat16, tag="tp")
                        nc.tensor.transpose(qtp, qr3[:, h, :], id_sb)
                        nc.vector.tensor_copy(out=qT_sb[:, h, qt, :], in_=qtp)

            mainps.close()

        # ================= phase C: attention =================
        with tc.tile_pool(name="sps", bufs=2, space="PSUM") as sps, \
             tc.tile_pool(name="yps", bufs=2, space="PSUM") as yps, \
             tc.tile_pool(name="pwork", bufs=4) as pwork:
            for h in range(H):
                kvh = h // REP
                for qt in range(NQT):
                    s = sps.tile([128, NKC, 128], dt.float32, tag="s")
                    for kc in range(NKC):
                        g = qt + kc
                        nc.tensor.matmul(s[:, kc, :], lhsT=kT_sb[:, kvh, g, :],
                                         rhs=qT_sb[:, h, qt, :],
                                         start=True, stop=True)
                    p = pwork.tile([128, NKC, 128], dt.bfloat16, tag="p")
                    nc.scalar.activation(
                        out=p.rearrange("p a n -> p (a n)"),
                        in_=s.rearrange("p a n -> p (a n)"), func=AF.Exp)
                    edge = bass.AP(tensor=p.tensor, offset=p.offset,
                                   ap=[p.ap[0], [(NKC - 1) * 128, 2], [1, 128]])
                    nc.gpsimd.tensor_mul(edge, edge, tri_sb)
                    y = yps.tile([128, 129], dt.float32, tag="y")
                    for kc in range(NKC):
                        g = qt + kc
                        nc.tensor.matmul(y, lhsT=p[:, kc, :],
                                         rhs=v_sb[:, g, kvh, :],
                                         start=(kc == 0), stop=(kc == NKC - 1))
                    z = work.tile([128, 1], dt.float32, tag="z")
                    nc.vector.tensor_sub(z, y[:, 128:129], npad_sb[:, qt:qt + 1])
                    nc.vector.reciprocal(out=z, in_=z)
                    nc.vector.tensor_scalar_mul(yN_sb[:, qt, h, :], y[:, 0:128], z)

        # ================= phase D: output projection =================
        with tc.tile_pool(name="tp2", bufs=2, space="PSUM") as tp2, \
             tc.tile_pool(name="ops", bufs=4, space="PSUM") as ops:
            for qt in range(NQT):
                yT = work.tile([128, H, 128], dt.bfloat16, tag="yT")
                for h in range(H):
                    ytp = tp2.tile([128, 128], dt.bfloat16, tag="ytp")
                    nc.tensor.transpose(ytp, yN_sb[:, qt, h, :], id_sb)
                    nc.vector.tensor_copy(out=yT[:, h, :], in_=ytp)
                for half in range(2):
                    o = ops.tile([128, 512], dt.float32, tag="o")
                    for h in range(H):
                        nc.tensor.matmul(o, lhsT=yT[:, h, :],
                                         rhs=wo_sb[:, h, bass.ts(half, 512)],
                                         start=(h == 0), stop=(h == H - 1))
                    osb = work.tile([128, 512], dt.float32, tag="osb")
                    nc.scalar.copy(out=osb, in_=o)
                    nc.sync.dma_start(
                        out=out_d[bass.ts(qt, 128), bass.ts(half, 512)], in_=osb)


# ---------------------------------------------------------------------------
# host side
# ---------------------------------------------------------------------------

def make_in_maps(x, ve, cos, sin, Wq, Wk, Wv, Wproj, Wg):
    """Build the 8 per-core input dicts (numpy, host-side prep)."""
    x = np.asarray(x, F32)
    ve = np.asarray(ve, F32)
    cos = np.asarray(cos, F32).reshape(T, 64)
    sin = np.asarray(sin, F32).reshape(T, 64)
    Wq = np.asarray(Wq, F32)
    Wk = np.asarray(Wk, F32)
    Wv = np.asarray(Wv, F32)
    Wproj = np.asarray(Wproj, F32)
    Wg = np.asarray(Wg, F32)

    wq = Wq.astype(BF16)
    wkv = np.concatenate([Wk, Wv], axis=1).astype(BF16)
    wo = Wproj.astype(BF16)
    wg = Wg.astype(BF16)
    ident = np.eye(128, dtype=BF16)

    # triangular masks in [k, q] layout
    kk = np.arange(128)[:, None]
    qq = np.arange(128)[None, :]
    tri = np.zeros((128, 2, 128), F32)
    tri[:, 0, :] = np.where(kk < qq, 0.0, 1.0)   # LEFT chunk (kc=0), mult mask
    tri[:, 1, :] = np.where(kk > qq, 0.0, 1.0)   # DIAG chunk (kc=8), mult mask
    tri = tri.reshape(128, 256).astype(BF16)

    in_maps = []
    for c in range(8):
        b, ck = divmod(c, 4)
        t0 = ck * RCHUNK
        es = t0 - WIN  # ext start (may be negative for chunk 0)
        pad = max(0, -es)

        def ext(a, fill_shape):
            out = np.zeros((E,) + fill_shape, F32)
            out[pad:] = a[es + pad: t0 + RCHUNK]
            return out

        x_e = ext(x[b], (C,))
        ve_e = ext(ve[b], (HKV * D,))
        cos_e = ext(cos, (64,))
        sin_e = ext(sin, (64,))

        npad = np.zeros((128, NQT), F32)
        if pad:
            kc = np.arange(NKC)[:, None]
            kl = np.arange(128)[None, :]
            r = np.arange(128)
            for qt in range(NQT):
                extpos = 128 * (qt + kc) + kl          # [9, 128]
                is_pad = extpos < pad
                for ri in r:
                    tri_ok = np.ones((NKC, 128), bool)
                    tri_ok[0] = kl[0] >= ri
                    tri_ok[NKC - 1] = kl[0] <= ri
                    npad[ri, qt] = np.sum(tri_ok & is_pad)

        in_maps.append({
            "xT": np.ascontiguousarray(x_e.T).astype(BF16),
            "wq": wq, "wkv": wkv, "wo": wo, "wg": wg,
            "ve2": (2.0 * ve_e).astype(BF16),
            "cs": np.concatenate([cos_e, cos_e, -sin_e, sin_e],
                                 axis=1).astype(BF16),
            "tri": tri, "npad": npad, "ident": ident,
        })
    return in_maps


_NC_CACHE = None


def kernel(x, ve, cos, sin, Wq, Wk, Wv, Wproj, Wg, window_size):
    assert int(window_size) == WIN
    global _NC_CACHE
    if _NC_CACHE is None:
        _NC_CACHE = build_nc()
    nc = _NC_CACHE
    in_maps = make_in_maps(x, ve, cos, sin, Wq, Wk, Wv, Wproj, Wg)
    res = bass_utils.run_bass_kernel_spmd(nc, in_maps, core_ids=list(range(8)))
    out = np.zeros((B, T, C), F32)
    for c in range(8):
        b, ck = divmod(c, 4)
        out[b, ck * RCHUNK:(ck + 1) * RCHUNK] = res.results[c]["out"]
    return out

